# revision 2
# baseline (speedup 1.0000x reference)
"""MoE block (nn_MoEBlock_40407052320888) on 8 Trainium2 NeuronCores.

Strategy (expert-parallel per the sharding hint):
- Router runs on host (tiny: 8192x2048x32 matmul = 0.1% of FLOPs); routing
  determines the shard layout, so it is part of input sharding.
- 32 experts sharded 4-per-core. Host gathers each expert's tokens
  (padded to capacity C), transposed to [D, C] so the device kernel needs
  zero on-chip transposes. Device does the heavy grouped SwiGLU GEMMs in
  bf16 with f32 PSUM accumulation.
- Shared-expert MLP is data-parallel: each core takes 1024 tokens.
- Host scatters routed outputs back (scatter-assign, weighted sum over K).
"""
import math
import sys

sys.path.insert(0, "/opt/trn_rl_repo")

import numpy as np
import ml_dtypes

import concourse.bacc as bacc
import concourse.mybir as mybir
import concourse.tile as tile
from concourse.bass_utils import run_bass_kernel_spmd

AF = mybir.ActivationFunctionType
ALU = mybir.AluOpType
BF16 = mybir.dt.bfloat16
F32 = mybir.dt.float32
bf16 = ml_dtypes.bfloat16

B, S, D, E, I = 4, 2048, 2048, 32, 1024
N = B * S
N_GROUP, TOPK_GROUP, TOP_K = 8, 4, 8
ROUTED_SCALE = 2.5
NCORES = 8
EPC = E // NCORES          # experts per core
TPC = N // NCORES          # tokens per core for the shared expert
KD = D // 128              # k-tiles over D
KI = I // 128              # k-tiles over I


def _chunks(C):
    out = [512] * (C // 512)
    if C % 512:
        out.append(C % 512)
    return out


def _build(C):
    """Bass program for one core: 4 experts (capacity C) + shared MLP."""
    nc = bacc.Bacc("TRN2", target_bir_lowering=False)
    xt_d = nc.dram_tensor("xt", [EPC, KD, 128, C], BF16, kind="ExternalInput")
    wgu_d = nc.dram_tensor("wgu", [EPC, KD, 128, 2 * I], BF16, kind="ExternalInput")
    wdn_d = nc.dram_tensor("wdn", [EPC, KI, 128, D], BF16, kind="ExternalInput")
    xts_d = nc.dram_tensor("xts", [KD, 128, TPC], BF16, kind="ExternalInput")
    wsg_d = nc.dram_tensor("wsg", [KD, 128, I], BF16, kind="ExternalInput")
    wsu_d = nc.dram_tensor("wsu", [KD, 128, I], BF16, kind="ExternalInput")
    wsd_d = nc.dram_tensor("wsd", [KI, 128, D], BF16, kind="ExternalInput")
    y_d = nc.dram_tensor("y", [EPC, C, D], F32, kind="ExternalOutput")
    ys_d = nc.dram_tensor("ys", [TPC, D], F32, kind="ExternalOutput")

    with tile.TileContext(nc) as tc:
        # ---------------- routed experts ----------------
        with (
            tc.tile_pool(name="wgu", bufs=24) as wgu_p,
            tc.tile_pool(name="wdn", bufs=10) as wdn_p,
            tc.tile_pool(name="xt", bufs=32) as xt_p,
            tc.tile_pool(name="h", bufs=4) as h_p,
            tc.tile_pool(name="inter", bufs=9) as i_p,
            tc.tile_pool(name="ob", bufs=4) as o_p,
            tc.tile_pool(name="ps1", bufs=3, space="PSUM") as ps1_p,
            tc.tile_pool(name="ps2", bufs=2, space="PSUM") as ps2_p,
        ):
            for e in range(EPC):
                wgu_sb = []
                for k in range(KD):
                    wt = wgu_p.tile([128, 2 * I], BF16, name="wgu")
                    nc.sync.dma_start(wt[:], wgu_d[e, k])
                    wgu_sb.append(wt)
                wdn_sb = []
                for k in range(KI):
                    wt = wdn_p.tile([128, D], BF16, name="wdn")
                    nc.sync.dma_start(wt[:], wdn_d[e, k])
                    wdn_sb.append(wt)

                cbase = 0
                for cs in _chunks(C):
                    xt_sb = []
                    for k in range(KD):
                        xtt = xt_p.tile([128, 512], BF16, name="xt")
                        nc.sync.dma_start(
                            xtt[:, :cs], xt_d[e, k, :, cbase:cbase + cs])
                        xt_sb.append(xtt)
                    inter = [i_p.tile([128, 512], BF16, name="inter") for _ in range(KI)]
                    for i in range(KI):
                        # h^T[m] = (W_gu[:, m*128:+128])^T @ x^T  for gate (m=i)
                        # and up (m=i+KI) halves of the 2I output axis.
                        pg = ps1_p.tile([128, 512], F32, name="ps1")
                        for k in range(KD):
                            nc.tensor.matmul(
                                pg[:, :cs],
                                wgu_sb[k][:, i * 128:(i + 1) * 128],
                                xt_sb[k][:, :cs],
                                start=(k == 0), stop=(k == KD - 1),
                            )
                        pu = ps1_p.tile([128, 512], F32, name="ps1")
                        for k in range(KD):
                            nc.tensor.matmul(
                                pu[:, :cs],
                                wgu_sb[k][:, (KI + i) * 128:(KI + i + 1) * 128],
                                xt_sb[k][:, :cs],
                                start=(k == 0), stop=(k == KD - 1),
                            )
                        hs = h_p.tile([128, 512], BF16, name="h")
                        nc.scalar.activation(hs[:, :cs], pg[:, :cs], AF.Silu)
                        hu = h_p.tile([128, 512], BF16, name="h")
                        nc.vector.tensor_copy(hu[:, :cs], pu[:, :cs])
                        nc.vector.tensor_tensor(
                            inter[i][:, :cs], hs[:, :cs], hu[:, :cs], ALU.mult
                        )
                    for m2 in range(cs // 128):
                        for n2 in range(D // 512):
                            ps2 = ps2_p.tile([128, 512], F32, name="ps2")
                            for k2 in range(KI):
                                nc.tensor.matmul(
                                    ps2[:],
                                    inter[k2][:, m2 * 128:(m2 + 1) * 128],
                                    wdn_sb[k2][:, n2 * 512:(n2 + 1) * 512],
                                    start=(k2 == 0), stop=(k2 == KI - 1),
                                )
                            ob = o_p.tile([128, 512], F32, name="ob")
                            if n2 % 2 == 0:
                                nc.scalar.copy(ob[:], ps2[:])
                            else:
                                nc.vector.tensor_copy(ob[:], ps2[:])
                            nc.sync.dma_start(
                                y_d[e, cbase + m2 * 128:cbase + (m2 + 1) * 128,
                                    n2 * 512:(n2 + 1) * 512],
                                ob[:],
                            )
                    cbase += cs

        # ---------------- shared expert (data-parallel) ----------------
        with (
            tc.tile_pool(name="swg", bufs=16) as swg_p,
            tc.tile_pool(name="swu", bufs=16) as swu_p,
            tc.tile_pool(name="swd", bufs=8) as swd_p,
            tc.tile_pool(name="sxt", bufs=16) as sxt_p,
            tc.tile_pool(name="sh", bufs=4) as sh_p,
            tc.tile_pool(name="sinter", bufs=9) as si_p,
            tc.tile_pool(name="sob", bufs=4) as so_p,
            tc.tile_pool(name="sps1", bufs=3, space="PSUM") as sps1_p,
            tc.tile_pool(name="sps2", bufs=2, space="PSUM") as sps2_p,
        ):
            wsg_sb, wsu_sb, xts_sb = [], [], []
            for k in range(KD):
                wt = swg_p.tile([128, I], BF16, name="swg")
                nc.sync.dma_start(wt[:], wsg_d[k])
                wsg_sb.append(wt)
                wt = swu_p.tile([128, I], BF16, name="swu")
                nc.sync.dma_start(wt[:], wsu_d[k])
                wsu_sb.append(wt)
                xtt = sxt_p.tile([128, TPC], BF16, name="sxt")
                nc.sync.dma_start(xtt[:], xts_d[k])
                xts_sb.append(xtt)
            wsd_sb = []
            for k in range(KI):
                wt = swd_p.tile([128, D], BF16, name="swd")
                nc.sync.dma_start(wt[:], wsd_d[k])
                wsd_sb.append(wt)

            for ch in range(TPC // 512):
                c0 = ch * 512
                inter = [si_p.tile([128, 512], BF16, name="sinter") for _ in range(KI)]
                for i in range(KI):
                    pg = sps1_p.tile([128, 512], F32, name="sps1")
                    for k in range(KD):
                        nc.tensor.matmul(
                            pg[:], wsg_sb[k][:, i * 128:(i + 1) * 128],
                            xts_sb[k][:, c0:c0 + 512],
                            start=(k == 0), stop=(k == KD - 1),
                        )
                    pu = sps1_p.tile([128, 512], F32, name="sps1")
                    for k in range(KD):
                        nc.tensor.matmul(
                            pu[:], wsu_sb[k][:, i * 128:(i + 1) * 128],
                            xts_sb[k][:, c0:c0 + 512],
                            start=(k == 0), stop=(k == KD - 1),
                        )
                    hs = sh_p.tile([128, 512], BF16, name="sh")
                    nc.scalar.activation(hs[:], pg[:], AF.Silu)
                    hu = sh_p.tile([128, 512], BF16, name="sh")
                    nc.vector.tensor_copy(hu[:], pu[:])
                    nc.vector.tensor_tensor(inter[i][:], hs[:], hu[:], ALU.mult)
                for m2 in range(4):
                    for n2 in range(D // 512):
                        ps2 = sps2_p.tile([128, 512], F32, name="sps2")
                        for k2 in range(KI):
                            nc.tensor.matmul(
                                ps2[:],
                                inter[k2][:, m2 * 128:(m2 + 1) * 128],
                                wsd_sb[k2][:, n2 * 512:(n2 + 1) * 512],
                                start=(k2 == 0), stop=(k2 == KI - 1),
                            )
                        ob = so_p.tile([128, 512], F32, name="sob")
                        if n2 % 2 == 0:
                            nc.scalar.copy(ob[:], ps2[:])
                        else:
                            nc.vector.tensor_copy(ob[:], ps2[:])
                        nc.sync.dma_start(
                            ys_d[c0 + m2 * 128:c0 + (m2 + 1) * 128,
                                 n2 * 512:(n2 + 1) * 512],
                            ob[:],
                        )
    nc.compile()
    return nc


_BUILD_CACHE = {}


def _get_nc(C):
    if C not in _BUILD_CACHE:
        _BUILD_CACHE[C] = _build(C)
    return _BUILD_CACHE[C]


def _route(x_flat, gate_w, e_bias):
    """Replicates the reference router in numpy (f32)."""
    logits = x_flat @ gate_w                      # [N, E]
    scores = 1.0 / (1.0 + np.exp(-logits))
    sfr = scores + e_bias
    epg = E // N_GROUP
    grouped = sfr.reshape(N, N_GROUP, epg)
    top2 = np.partition(grouped, epg - 2, axis=2)[:, :, epg - 2:].sum(2)
    topg = np.argsort(-top2, axis=1, kind="stable")[:, :TOPK_GROUP]
    gmask = np.zeros((N, N_GROUP), bool)
    gmask[np.arange(N)[:, None], topg] = True
    emask = np.repeat(gmask, epg, axis=1)
    masked = np.where(emask, sfr, -np.inf)
    topk_idx = np.argsort(-masked, axis=1, kind="stable")[:, :TOP_K].astype(np.int32)
    topk_w = np.take_along_axis(scores, topk_idx, axis=1)
    topk_w = topk_w / (topk_w.sum(-1, keepdims=True) + 1e-20) * ROUTED_SCALE
    return topk_idx, topk_w, scores


def _prep_in_maps(x_flat, topk_idx, gate_up, down, shared_gate, shared_up,
                  shared_down):
    flat = topk_idx.reshape(-1).astype(np.int64)
    order = np.argsort(flat, kind="stable")
    counts = np.bincount(flat, minlength=E)
    starts = np.zeros(E + 1, np.int64)
    np.cumsum(counts, out=starts[1:])
    C = max(512, int(math.ceil(counts.max() / 128)) * 128)

    x_bf = x_flat.astype(bf16)
    wsg = np.ascontiguousarray(shared_gate.astype(bf16)).reshape(KD, 128, I)
    wsu = np.ascontiguousarray(shared_up.astype(bf16)).reshape(KD, 128, I)
    wsd = np.ascontiguousarray(shared_down.astype(bf16)).reshape(KI, 128, D)

    in_maps = []
    tids_all = []
    for c in range(NCORES):
        xt = np.zeros((EPC, KD, 128, C), bf16)
        for s in range(EPC):
            e = c * EPC + s
            tids = order[starts[e]:starts[e + 1]] // TOP_K
            tids_all.append(tids)
            xt[s].reshape(D, C)[:, :counts[e]] = x_bf[tids].T
        wgu = gate_up[c * EPC:(c + 1) * EPC].astype(bf16).reshape(
            EPC, KD, 128, 2 * I)
        wdn = down[c * EPC:(c + 1) * EPC].astype(bf16).reshape(EPC, KI, 128, D)
        xts = np.ascontiguousarray(x_bf[c * TPC:(c + 1) * TPC].T).reshape(
            KD, 128, TPC)
        in_maps.append({
            "xt": xt, "wgu": wgu, "wdn": wdn, "xts": xts,
            "wsg": wsg, "wsu": wsu, "wsd": wsd,
        })
    return in_maps, order, counts, starts, C


def kernel(x, gate_w, e_bias, gate_up, down, shared_gate, shared_up,
           shared_down):
    x = np.asarray(x, np.float32)
    gate_w = np.asarray(gate_w, np.float32)
    e_bias = np.asarray(e_bias, np.float32)
    gate_up = np.asarray(gate_up, np.float32)
    down = np.asarray(down, np.float32)
    shared_gate = np.asarray(shared_gate, np.float32)
    shared_up = np.asarray(shared_up, np.float32)
    shared_down = np.asarray(shared_down, np.float32)

    x_flat = x.reshape(N, D)
    topk_idx, topk_w, scores = _route(x_flat, gate_w, e_bias)
    in_maps, order, counts, starts, C = _prep_in_maps(
        x_flat, topk_idx, gate_up, down, shared_gate, shared_up, shared_down)

    nc = _get_nc(C)
    res = run_bass_kernel_spmd(nc, in_maps, core_ids=list(range(NCORES)))
    results = res.results

    sorted_out = np.empty((N * TOP_K, D), np.float32)
    for e in range(E):
        c, s = e // EPC, e % EPC
        sorted_out[starts[e]:starts[e + 1]] = results[c]["y"][s][:counts[e]]
    w_sorted = topk_w.reshape(-1)[order].astype(np.float32)
    sorted_out *= w_sorted[:, None]
    unsorted = np.empty_like(sorted_out)
    unsorted[order] = sorted_out
    routed = unsorted.reshape(N, TOP_K, D).sum(1)

    shared = np.concatenate([results[c]["ys"] for c in range(NCORES)], 0)
    out = (routed + shared).reshape(B, S, D)
    return out, topk_idx, scores


# revision 7
# speedup vs baseline: 1.1639x; 1.1639x over previous
"""MoE block (nn_MoEBlock_40407052320888) on 8 Trainium2 NeuronCores.

Strategy (expert-parallel per the sharding hint):
- Router runs on host (tiny: 8192x2048x32 matmul = 0.1% of FLOPs); routing
  determines the shard layout, so it is part of input sharding.
- 32 experts sharded 4-per-core. Host gathers each expert's tokens
  (padded to capacity C), transposed to [D, C] so the device kernel needs
  zero on-chip transposes. Device does the heavy grouped SwiGLU GEMMs in
  bf16 with f32 PSUM accumulation.
- Shared-expert MLP is data-parallel: each core takes 1024 tokens.
- Host scatters routed outputs back (scatter-assign, weighted sum over K).
"""
import math
import sys

sys.path.insert(0, "/opt/trn_rl_repo")

import numpy as np
import ml_dtypes

import concourse.bacc as bacc
import concourse.mybir as mybir
import concourse.tile as tile
from concourse.bass_utils import run_bass_kernel_spmd

AF = mybir.ActivationFunctionType
ALU = mybir.AluOpType
BF16 = mybir.dt.bfloat16
F32 = mybir.dt.float32
bf16 = ml_dtypes.bfloat16

B, S, D, E, I = 4, 2048, 2048, 32, 1024
N = B * S
N_GROUP, TOPK_GROUP, TOP_K = 8, 4, 8
ROUTED_SCALE = 2.5
NCORES = 8
EPC = E // NCORES          # experts per core
TPC = N // NCORES          # tokens per core for the shared expert
KD = D // 128              # k-tiles over D
KI = I // 128              # k-tiles over I


def _chunks(C):
    out = [512] * (C // 512)
    if C % 512:
        out.append(C % 512)
    return out


def _build(C):
    """Bass program for one core: 4 experts (capacity C) + shared MLP."""
    nc = bacc.Bacc("TRN2", target_bir_lowering=False)
    xt_d = nc.dram_tensor("xt", [EPC, KD, 128, C], BF16, kind="ExternalInput")
    wgu_d = nc.dram_tensor("wgu", [EPC, KD, 128, 2 * I], BF16, kind="ExternalInput")
    wdn_d = nc.dram_tensor("wdn", [EPC, KI, 128, D], BF16, kind="ExternalInput")
    xts_d = nc.dram_tensor("xts", [KD, 128, TPC], BF16, kind="ExternalInput")
    wsg_d = nc.dram_tensor("wsg", [KD, 128, I], BF16, kind="ExternalInput")
    wsu_d = nc.dram_tensor("wsu", [KD, 128, I], BF16, kind="ExternalInput")
    wsd_d = nc.dram_tensor("wsd", [KI, 128, D], BF16, kind="ExternalInput")
    y_d = nc.dram_tensor("y", [EPC, C, D], BF16, kind="ExternalOutput")
    ys_d = nc.dram_tensor("ys", [TPC, D], BF16, kind="ExternalOutput")

    def swiglu_gemms(wg_sb, g_off, wu_sb, u_off, xin, xoff, cs, wd_sb,
                     out_d, obase, i_p, h_p, ps1_p, ps2_p, st_p):
        """h = silu(Wg^T x)*(Wu^T x); out = h^T Wd ; writes [cs, D] to
        out_d[obase:obase+cs]. xin: list of KD sbuf tiles [128, >=xoff+cs]."""
        inter = [i_p.tile([128, 512], BF16, name="inter") for _ in range(KI)]
        for i in range(KI):
            pg = ps1_p.tile([128, 512], F32, name="ps1")
            for k in range(KD):
                nc.tensor.matmul(
                    pg[:, :cs],
                    wg_sb[k][:, g_off + i * 128:g_off + (i + 1) * 128],
                    xin[k][:, xoff:xoff + cs],
                    start=(k == 0), stop=(k == KD - 1))
            pu = ps1_p.tile([128, 512], F32, name="ps1")
            for k in range(KD):
                nc.tensor.matmul(
                    pu[:, :cs],
                    wu_sb[k][:, u_off + i * 128:u_off + (i + 1) * 128],
                    xin[k][:, xoff:xoff + cs],
                    start=(k == 0), stop=(k == KD - 1))
            hs = h_p.tile([128, 512], BF16, name="h")
            nc.scalar.activation(hs[:, :cs], pg[:, :cs], AF.Silu)
            hu = h_p.tile([128, 512], BF16, name="h")
            nc.vector.tensor_copy(hu[:, :cs], pu[:, :cs])
            nc.vector.tensor_tensor(
                inter[i][:, :cs], hs[:, :cs], hu[:, :cs], ALU.mult)
        for m2 in range(cs // 128):
            st = st_p.tile([128, D], BF16, name="st")
            for n2 in range(D // 512):
                ps2 = ps2_p.tile([128, 512], F32, name="ps2")
                for k2 in range(KI):
                    nc.tensor.matmul(
                        ps2[:], inter[k2][:, m2 * 128:(m2 + 1) * 128],
                        wd_sb[k2][:, n2 * 512:(n2 + 1) * 512],
                        start=(k2 == 0), stop=(k2 == KI - 1))
                if n2 % 2 == 0:
                    nc.scalar.copy(st[:, n2 * 512:(n2 + 1) * 512], ps2[:])
                else:
                    nc.vector.tensor_copy(st[:, n2 * 512:(n2 + 1) * 512], ps2[:])
            nc.scalar.dma_start(
                out_d[obase + m2 * 128:obase + (m2 + 1) * 128, :], st[:])

    with tile.TileContext(nc) as tc:
        # ---------------- routed experts ----------------
        with (
            tc.tile_pool(name="wgu", bufs=20) as wgu_p,
            tc.tile_pool(name="wdn", bufs=8) as wdn_p,
            tc.tile_pool(name="xt", bufs=16) as xt_p,
            tc.tile_pool(name="h", bufs=3) as h_p,
            tc.tile_pool(name="inter", bufs=8) as i_p,
            tc.tile_pool(name="st", bufs=2) as st_p,
            tc.tile_pool(name="ps1", bufs=3, space="PSUM") as ps1_p,
            tc.tile_pool(name="ps2", bufs=2, space="PSUM") as ps2_p,
        ):
            for e in range(EPC):
                wgu_sb = []
                xt_sb = []
                for k in range(KD):
                    wt = wgu_p.tile([128, 2 * I], BF16, name="wgu")
                    nc.sync.dma_start(wt[:], wgu_d[e, k])
                    wgu_sb.append(wt)
                    xtt = xt_p.tile([128, C], BF16, name="xt")
                    nc.sync.dma_start(xtt[:], xt_d[e, k])
                    xt_sb.append(xtt)
                wdn_sb = []
                for k in range(KI):
                    wt = wdn_p.tile([128, D], BF16, name="wdn")
                    nc.sync.dma_start(wt[:], wdn_d[e, k])
                    wdn_sb.append(wt)
                cbase = 0
                for cs in _chunks(C):
                    swiglu_gemms(wgu_sb, 0, wgu_sb, I, xt_sb,
                                 cbase, cs, wdn_sb, y_d[e], cbase,
                                 i_p, h_p, ps1_p, ps2_p, st_p)
                    cbase += cs

        # ---------------- shared expert (data-parallel) ----------------
        with (
            tc.tile_pool(name="swg", bufs=16) as swg_p,
            tc.tile_pool(name="swu", bufs=16) as swu_p,
            tc.tile_pool(name="swd", bufs=8) as swd_p,
            tc.tile_pool(name="sxt", bufs=16) as sxt_p,
            tc.tile_pool(name="sh", bufs=3) as sh_p,
            tc.tile_pool(name="sinter", bufs=8) as si_p,
            tc.tile_pool(name="sst", bufs=2) as sst_p,
            tc.tile_pool(name="sps1", bufs=3, space="PSUM") as sps1_p,
            tc.tile_pool(name="sps2", bufs=2, space="PSUM") as sps2_p,
        ):
            wsg_sb, wsu_sb, xts_sb = [], [], []
            for k in range(KD):
                wt = swg_p.tile([128, I], BF16, name="swg")
                nc.sync.dma_start(wt[:], wsg_d[k])
                wsg_sb.append(wt)
                wt = swu_p.tile([128, I], BF16, name="swu")
                nc.sync.dma_start(wt[:], wsu_d[k])
                wsu_sb.append(wt)
                xtt = sxt_p.tile([128, TPC], BF16, name="sxt")
                nc.sync.dma_start(xtt[:], xts_d[k])
                xts_sb.append(xtt)
            wsd_sb = []
            for k in range(KI):
                wt = swd_p.tile([128, D], BF16, name="swd")
                nc.sync.dma_start(wt[:], wsd_d[k])
                wsd_sb.append(wt)
            for ch in range(TPC // 512):
                swiglu_gemms(wsg_sb, 0, wsu_sb, 0, xts_sb, ch * 512, 512,
                             wsd_sb, ys_d, ch * 512,
                             si_p, sh_p, sps1_p, sps2_p, sst_p)
    nc.compile()
    return nc


_BUILD_CACHE = {}


def _get_nc(C):
    if C not in _BUILD_CACHE:
        _BUILD_CACHE[C] = _build(C)
    return _BUILD_CACHE[C]


def _route(x_flat, gate_w, e_bias):
    """Replicates the reference router in numpy (f32)."""
    logits = x_flat @ gate_w                      # [N, E]
    scores = 1.0 / (1.0 + np.exp(-logits))
    sfr = scores + e_bias
    epg = E // N_GROUP
    grouped = sfr.reshape(N, N_GROUP, epg)
    top2 = np.partition(grouped, epg - 2, axis=2)[:, :, epg - 2:].sum(2)
    topg = np.argsort(-top2, axis=1, kind="stable")[:, :TOPK_GROUP]
    gmask = np.zeros((N, N_GROUP), bool)
    gmask[np.arange(N)[:, None], topg] = True
    emask = np.repeat(gmask, epg, axis=1)
    masked = np.where(emask, sfr, -np.inf)
    topk_idx = np.argsort(-masked, axis=1, kind="stable")[:, :TOP_K].astype(np.int32)
    topk_w = np.take_along_axis(scores, topk_idx, axis=1)
    topk_w = topk_w / (topk_w.sum(-1, keepdims=True) + 1e-20) * ROUTED_SCALE
    return topk_idx, topk_w, scores


def _prep_in_maps(x_flat, topk_idx, gate_up, down, shared_gate, shared_up,
                  shared_down):
    flat = topk_idx.reshape(-1).astype(np.int64)
    order = np.argsort(flat, kind="stable")
    counts = np.bincount(flat, minlength=E)
    starts = np.zeros(E + 1, np.int64)
    np.cumsum(counts, out=starts[1:])
    C = max(512, int(math.ceil(counts.max() / 128)) * 128)

    x_bf = x_flat.astype(bf16)
    wsg = np.ascontiguousarray(shared_gate.astype(bf16)).reshape(KD, 128, I)
    wsu = np.ascontiguousarray(shared_up.astype(bf16)).reshape(KD, 128, I)
    wsd = np.ascontiguousarray(shared_down.astype(bf16)).reshape(KI, 128, D)

    in_maps = []
    tids_all = []
    for c in range(NCORES):
        xt = np.zeros((EPC, KD, 128, C), bf16)
        for s in range(EPC):
            e = c * EPC + s
            tids = order[starts[e]:starts[e + 1]] // TOP_K
            tids_all.append(tids)
            xt[s].reshape(D, C)[:, :counts[e]] = x_bf[tids].T
        wgu = gate_up[c * EPC:(c + 1) * EPC].astype(bf16).reshape(
            EPC, KD, 128, 2 * I)
        wdn = down[c * EPC:(c + 1) * EPC].astype(bf16).reshape(EPC, KI, 128, D)
        xts = np.ascontiguousarray(x_bf[c * TPC:(c + 1) * TPC].T).reshape(
            KD, 128, TPC)
        in_maps.append({
            "xt": xt, "wgu": wgu, "wdn": wdn, "xts": xts,
            "wsg": wsg, "wsu": wsu, "wsd": wsd,
        })
    return in_maps, order, counts, starts, C


def kernel(x, gate_w, e_bias, gate_up, down, shared_gate, shared_up,
           shared_down):
    x = np.asarray(x, np.float32)
    gate_w = np.asarray(gate_w, np.float32)
    e_bias = np.asarray(e_bias, np.float32)
    gate_up = np.asarray(gate_up, np.float32)
    down = np.asarray(down, np.float32)
    shared_gate = np.asarray(shared_gate, np.float32)
    shared_up = np.asarray(shared_up, np.float32)
    shared_down = np.asarray(shared_down, np.float32)

    x_flat = x.reshape(N, D)
    topk_idx, topk_w, scores = _route(x_flat, gate_w, e_bias)
    in_maps, order, counts, starts, C = _prep_in_maps(
        x_flat, topk_idx, gate_up, down, shared_gate, shared_up, shared_down)

    nc = _get_nc(C)
    res = run_bass_kernel_spmd(nc, in_maps, core_ids=list(range(NCORES)))
    results = res.results

    sorted_out = np.empty((N * TOP_K, D), np.float32)
    for e in range(E):
        c, s = e // EPC, e % EPC
        sorted_out[starts[e]:starts[e + 1]] = results[c]["y"][s][:counts[e]]
    w_sorted = topk_w.reshape(-1)[order].astype(np.float32)
    sorted_out *= w_sorted[:, None]
    unsorted = np.empty_like(sorted_out)
    unsorted[order] = sorted_out
    routed = unsorted.reshape(N, TOP_K, D).sum(1)

    shared = np.concatenate(
        [results[c]["ys"].astype(np.float32) for c in range(NCORES)], 0)
    out = (routed + shared).reshape(B, S, D)
    return out, topk_idx, scores


# revision 8
# speedup vs baseline: 1.8510x; 1.5903x over previous
"""MoE block (nn_MoEBlock_40407052320888) on 8 Trainium2 NeuronCores.

Strategy (expert-parallel per the sharding hint):
- Router runs on host (tiny: 8192x2048x32 matmul = 0.1% of FLOPs); routing
  determines the shard layout, so it is part of input sharding.
- 32 experts sharded 4-per-core. Host gathers each expert's tokens
  (padded to capacity C), transposed to [D, C] so the device kernel needs
  zero on-chip transposes. Device does the heavy grouped SwiGLU GEMMs in
  bf16 with f32 PSUM accumulation.
- Shared-expert MLP is data-parallel: each core takes 1024 tokens.
- Host scatters routed outputs back (scatter-assign, weighted sum over K).
"""
import math
import sys

sys.path.insert(0, "/opt/trn_rl_repo")

import numpy as np
import ml_dtypes

import concourse.bacc as bacc
import concourse.mybir as mybir
import concourse.tile as tile
from concourse.bass_utils import run_bass_kernel_spmd

AF = mybir.ActivationFunctionType
ALU = mybir.AluOpType
BF16 = mybir.dt.bfloat16
F32 = mybir.dt.float32
bf16 = ml_dtypes.bfloat16

B, S, D, E, I = 4, 2048, 2048, 32, 1024
N = B * S
N_GROUP, TOPK_GROUP, TOP_K = 8, 4, 8
ROUTED_SCALE = 2.5
NCORES = 8
EPC = E // NCORES          # experts per core
TPC = N // NCORES          # tokens per core for the shared expert
KD = D // 128              # k-tiles over D
KI = I // 128              # k-tiles over I


def _chunks(C):
    out = [512] * (C // 512)
    if C % 512:
        out.append(C % 512)
    return out


def _build(C):
    """Bass program for one core: 4 experts (capacity C) + shared MLP."""
    nc = bacc.Bacc("TRN2", target_bir_lowering=False)
    xt_d = nc.dram_tensor("xt", [EPC, KD, 128, C], BF16, kind="ExternalInput")
    wgu_d = nc.dram_tensor("wgu", [EPC, KD, 128, 2 * I], BF16, kind="ExternalInput")
    wdn_d = nc.dram_tensor("wdn", [EPC, KI, 128, D], BF16, kind="ExternalInput")
    xts_d = nc.dram_tensor("xts", [KD, 128, TPC], BF16, kind="ExternalInput")
    wsg_d = nc.dram_tensor("wsg", [KD, 128, I], BF16, kind="ExternalInput")
    wsu_d = nc.dram_tensor("wsu", [KD, 128, I], BF16, kind="ExternalInput")
    wsd_d = nc.dram_tensor("wsd", [KI, 128, D], BF16, kind="ExternalInput")
    y_d = nc.dram_tensor("y", [EPC, C, D], BF16, kind="ExternalOutput")
    ys_d = nc.dram_tensor("ys", [TPC, D], BF16, kind="ExternalOutput")

    def swiglu_gemms(wg_sb, g_off, wu_sb, u_off, xin, xoff, cs, wd_sb,
                     out_d, obase, i_p, h_p, ps1_p, ps2_p, st_p):
        """h = silu(Wg^T x)*(Wu^T x); out = h^T Wd ; writes [cs, D] to
        out_d[obase:obase+cs]. xin: list of KD sbuf tiles [128, >=xoff+cs]."""
        inter = [i_p.tile([128, 512], BF16, name="inter") for _ in range(KI)]
        for i in range(KI):
            pg = ps1_p.tile([128, 512], F32, name="ps1")
            for k in range(KD):
                nc.tensor.matmul(
                    pg[:, :cs],
                    wg_sb[k][:, g_off + i * 128:g_off + (i + 1) * 128],
                    xin[k][:, xoff:xoff + cs],
                    start=(k == 0), stop=(k == KD - 1))
            pu = ps1_p.tile([128, 512], F32, name="ps1")
            for k in range(KD):
                nc.tensor.matmul(
                    pu[:, :cs],
                    wu_sb[k][:, u_off + i * 128:u_off + (i + 1) * 128],
                    xin[k][:, xoff:xoff + cs],
                    start=(k == 0), stop=(k == KD - 1))
            hs = h_p.tile([128, 512], BF16, name="h")
            nc.scalar.activation(hs[:, :cs], pg[:, :cs], AF.Silu)
            hu = h_p.tile([128, 512], BF16, name="h")
            nc.vector.tensor_copy(hu[:, :cs], pu[:, :cs])
            nc.vector.tensor_tensor(
                inter[i][:, :cs], hs[:, :cs], hu[:, :cs], ALU.mult)
        for m2 in range(cs // 128):
            st = st_p.tile([128, D], BF16, name="st")
            for n2 in range(D // 512):
                ps2 = ps2_p.tile([128, 512], F32, name="ps2")
                for k2 in range(KI):
                    nc.tensor.matmul(
                        ps2[:], inter[k2][:, m2 * 128:(m2 + 1) * 128],
                        wd_sb[k2][:, n2 * 512:(n2 + 1) * 512],
                        start=(k2 == 0), stop=(k2 == KI - 1))
                if n2 % 2 == 0:
                    nc.scalar.copy(st[:, n2 * 512:(n2 + 1) * 512], ps2[:])
                else:
                    nc.vector.tensor_copy(st[:, n2 * 512:(n2 + 1) * 512], ps2[:])
            nc.sync.dma_start(
                out_d[obase + m2 * 128:obase + (m2 + 1) * 128, :], st[:])

    with tile.TileContext(nc) as tc:
        # ---------------- routed experts ----------------
        with (
            tc.tile_pool(name="wgu", bufs=20) as wgu_p,
            tc.tile_pool(name="wdn", bufs=8) as wdn_p,
            tc.tile_pool(name="xt", bufs=16) as xt_p,
            tc.tile_pool(name="h", bufs=3) as h_p,
            tc.tile_pool(name="inter", bufs=8) as i_p,
            tc.tile_pool(name="st", bufs=2) as st_p,
            tc.tile_pool(name="ps1", bufs=3, space="PSUM") as ps1_p,
            tc.tile_pool(name="ps2", bufs=2, space="PSUM") as ps2_p,
        ):
            for e in range(EPC):
                wgu_sb = []
                xt_sb = []
                for k in range(KD):
                    wt = wgu_p.tile([128, 2 * I], BF16, name="wgu")
                    nc.sync.dma_start(wt[:], wgu_d[e, k])
                    wgu_sb.append(wt)
                    xtt = xt_p.tile([128, C], BF16, name="xt")
                    nc.sync.dma_start(xtt[:], xt_d[e, k])
                    xt_sb.append(xtt)
                wdn_sb = []
                for k in range(KI):
                    wt = wdn_p.tile([128, D], BF16, name="wdn")
                    nc.sync.dma_start(wt[:], wdn_d[e, k])
                    wdn_sb.append(wt)
                cbase = 0
                for cs in _chunks(C):
                    swiglu_gemms(wgu_sb, 0, wgu_sb, I, xt_sb,
                                 cbase, cs, wdn_sb, y_d[e], cbase,
                                 i_p, h_p, ps1_p, ps2_p, st_p)
                    cbase += cs

        # ---------------- shared expert (data-parallel) ----------------
        with (
            tc.tile_pool(name="swg", bufs=16) as swg_p,
            tc.tile_pool(name="swu", bufs=16) as swu_p,
            tc.tile_pool(name="swd", bufs=8) as swd_p,
            tc.tile_pool(name="sxt", bufs=16) as sxt_p,
            tc.tile_pool(name="sh", bufs=3) as sh_p,
            tc.tile_pool(name="sinter", bufs=8) as si_p,
            tc.tile_pool(name="sst", bufs=2) as sst_p,
            tc.tile_pool(name="sps1", bufs=3, space="PSUM") as sps1_p,
            tc.tile_pool(name="sps2", bufs=2, space="PSUM") as sps2_p,
        ):
            wsg_sb, wsu_sb, xts_sb = [], [], []
            for k in range(KD):
                wt = swg_p.tile([128, I], BF16, name="swg")
                nc.sync.dma_start(wt[:], wsg_d[k])
                wsg_sb.append(wt)
                wt = swu_p.tile([128, I], BF16, name="swu")
                nc.sync.dma_start(wt[:], wsu_d[k])
                wsu_sb.append(wt)
                xtt = sxt_p.tile([128, TPC], BF16, name="sxt")
                nc.sync.dma_start(xtt[:], xts_d[k])
                xts_sb.append(xtt)
            wsd_sb = []
            for k in range(KI):
                wt = swd_p.tile([128, D], BF16, name="swd")
                nc.sync.dma_start(wt[:], wsd_d[k])
                wsd_sb.append(wt)
            for ch in range(TPC // 512):
                swiglu_gemms(wsg_sb, 0, wsu_sb, 0, xts_sb, ch * 512, 512,
                             wsd_sb, ys_d, ch * 512,
                             si_p, sh_p, sps1_p, sps2_p, sst_p)
    nc.compile()
    return nc


_BUILD_CACHE = {}


def _get_nc(C):
    if C not in _BUILD_CACHE:
        _BUILD_CACHE[C] = _build(C)
    return _BUILD_CACHE[C]


def _route(x_flat, gate_w, e_bias):
    """Replicates the reference router in numpy (f32)."""
    logits = x_flat @ gate_w                      # [N, E]
    scores = 1.0 / (1.0 + np.exp(-logits))
    sfr = scores + e_bias
    epg = E // N_GROUP
    grouped = sfr.reshape(N, N_GROUP, epg)
    top2 = np.partition(grouped, epg - 2, axis=2)[:, :, epg - 2:].sum(2)
    topg = np.argsort(-top2, axis=1, kind="stable")[:, :TOPK_GROUP]
    gmask = np.zeros((N, N_GROUP), bool)
    gmask[np.arange(N)[:, None], topg] = True
    emask = np.repeat(gmask, epg, axis=1)
    masked = np.where(emask, sfr, -np.inf)
    topk_idx = np.argsort(-masked, axis=1, kind="stable")[:, :TOP_K].astype(np.int32)
    topk_w = np.take_along_axis(scores, topk_idx, axis=1)
    topk_w = topk_w / (topk_w.sum(-1, keepdims=True) + 1e-20) * ROUTED_SCALE
    return topk_idx, topk_w, scores


def _prep_in_maps(x_flat, topk_idx, gate_up, down, shared_gate, shared_up,
                  shared_down):
    flat = topk_idx.reshape(-1).astype(np.int64)
    order = np.argsort(flat, kind="stable")
    counts = np.bincount(flat, minlength=E)
    starts = np.zeros(E + 1, np.int64)
    np.cumsum(counts, out=starts[1:])
    C = max(512, int(math.ceil(counts.max() / 128)) * 128)

    x_bf = x_flat.astype(bf16)
    wsg = np.ascontiguousarray(shared_gate.astype(bf16)).reshape(KD, 128, I)
    wsu = np.ascontiguousarray(shared_up.astype(bf16)).reshape(KD, 128, I)
    wsd = np.ascontiguousarray(shared_down.astype(bf16)).reshape(KI, 128, D)

    in_maps = []
    tids_all = []
    for c in range(NCORES):
        xt = np.zeros((EPC, KD, 128, C), bf16)
        for s in range(EPC):
            e = c * EPC + s
            tids = order[starts[e]:starts[e + 1]] // TOP_K
            tids_all.append(tids)
            xt[s].reshape(D, C)[:, :counts[e]] = x_bf[tids].T
        wgu = gate_up[c * EPC:(c + 1) * EPC].astype(bf16).reshape(
            EPC, KD, 128, 2 * I)
        wdn = down[c * EPC:(c + 1) * EPC].astype(bf16).reshape(EPC, KI, 128, D)
        xts = np.ascontiguousarray(x_bf[c * TPC:(c + 1) * TPC].T).reshape(
            KD, 128, TPC)
        in_maps.append({
            "xt": xt, "wgu": wgu, "wdn": wdn, "xts": xts,
            "wsg": wsg, "wsu": wsu, "wsd": wsd,
        })
    return in_maps, order, counts, starts, C


def kernel(x, gate_w, e_bias, gate_up, down, shared_gate, shared_up,
           shared_down):
    x = np.asarray(x, np.float32)
    gate_w = np.asarray(gate_w, np.float32)
    e_bias = np.asarray(e_bias, np.float32)
    gate_up = np.asarray(gate_up, np.float32)
    down = np.asarray(down, np.float32)
    shared_gate = np.asarray(shared_gate, np.float32)
    shared_up = np.asarray(shared_up, np.float32)
    shared_down = np.asarray(shared_down, np.float32)

    x_flat = x.reshape(N, D)
    topk_idx, topk_w, scores = _route(x_flat, gate_w, e_bias)
    in_maps, order, counts, starts, C = _prep_in_maps(
        x_flat, topk_idx, gate_up, down, shared_gate, shared_up, shared_down)

    nc = _get_nc(C)
    res = run_bass_kernel_spmd(nc, in_maps, core_ids=list(range(NCORES)))
    results = res.results

    sorted_out = np.empty((N * TOP_K, D), np.float32)
    for e in range(E):
        c, s = e // EPC, e % EPC
        sorted_out[starts[e]:starts[e + 1]] = results[c]["y"][s][:counts[e]]
    w_sorted = topk_w.reshape(-1)[order].astype(np.float32)
    sorted_out *= w_sorted[:, None]
    unsorted = np.empty_like(sorted_out)
    unsorted[order] = sorted_out
    routed = unsorted.reshape(N, TOP_K, D).sum(1)

    shared = np.concatenate(
        [results[c]["ys"].astype(np.float32) for c in range(NCORES)], 0)
    out = (routed + shared).reshape(B, S, D)
    return out, topk_idx, scores


# revision 9
# speedup vs baseline: 1.9675x; 1.0629x over previous
"""MoE block (nn_MoEBlock_40407052320888) on 8 Trainium2 NeuronCores.

Strategy (expert-parallel per the sharding hint):
- Router runs on host (tiny: 8192x2048x32 matmul = 0.1% of FLOPs); routing
  determines the shard layout, so it is part of input sharding.
- 32 experts sharded 4-per-core. Host gathers each expert's tokens
  (padded to capacity C), transposed to [D, C] so the device kernel needs
  zero on-chip transposes. Device does the heavy grouped SwiGLU GEMMs in
  bf16 with f32 PSUM accumulation.
- Shared-expert MLP is data-parallel: each core takes 1024 tokens.
- Host scatters routed outputs back (scatter-assign, weighted sum over K).
"""
import math
import sys

sys.path.insert(0, "/opt/trn_rl_repo")

import numpy as np
import ml_dtypes

import concourse.bacc as bacc
import concourse.mybir as mybir
import concourse.tile as tile
from concourse.bass_utils import run_bass_kernel_spmd

AF = mybir.ActivationFunctionType
ALU = mybir.AluOpType
BF16 = mybir.dt.bfloat16
F32 = mybir.dt.float32
bf16 = ml_dtypes.bfloat16

B, S, D, E, I = 4, 2048, 2048, 32, 1024
N = B * S
N_GROUP, TOPK_GROUP, TOP_K = 8, 4, 8
ROUTED_SCALE = 2.5
NCORES = 8
EPC = E // NCORES          # experts per core
TPC = N // NCORES          # tokens per core for the shared expert
KD = D // 128              # k-tiles over D
KI = I // 128              # k-tiles over I


def _chunks(C):
    out = [512] * (C // 512)
    if C % 512:
        out.append(C % 512)
    return out


def _build(C):
    """Bass program for one core: 4 experts (capacity C) + shared MLP."""
    nc = bacc.Bacc("TRN2", target_bir_lowering=False)
    xt_d = nc.dram_tensor("xt", [EPC, KD, 128, C], BF16, kind="ExternalInput")
    wgu_d = nc.dram_tensor("wgu", [EPC, KD, 128, 2 * I], BF16, kind="ExternalInput")
    wdn_d = nc.dram_tensor("wdn", [EPC, KI, 128, D], BF16, kind="ExternalInput")
    xts_d = nc.dram_tensor("xts", [KD, 128, TPC], BF16, kind="ExternalInput")
    wsg_d = nc.dram_tensor("wsg", [KD, 128, I], BF16, kind="ExternalInput")
    wsu_d = nc.dram_tensor("wsu", [KD, 128, I], BF16, kind="ExternalInput")
    wsd_d = nc.dram_tensor("wsd", [KI, 128, D], BF16, kind="ExternalInput")
    y_d = nc.dram_tensor("y", [EPC, C, D], BF16, kind="ExternalOutput")
    ys_d = nc.dram_tensor("ys", [TPC, D], BF16, kind="ExternalOutput")

    def swiglu_gemms(wg_sb, g_off, wu_sb, u_off, xin, xoff, cs, wd_sb,
                     out_d, obase, i_p, h_p, ps1_p, ps2_p, st_p):
        """h = silu(Wg^T x)*(Wu^T x); out = h^T Wd ; writes [cs, D] to
        out_d[obase:obase+cs]. xin: list of KD sbuf tiles [128, >=xoff+cs]."""
        inter = [i_p.tile([128, 512], BF16, name="inter") for _ in range(KI)]
        for i in range(KI):
            pg = ps1_p.tile([128, 512], F32, name="ps1")
            for k in range(KD):
                nc.tensor.matmul(
                    pg[:, :cs],
                    wg_sb[k][:, g_off + i * 128:g_off + (i + 1) * 128],
                    xin[k][:, xoff:xoff + cs],
                    start=(k == 0), stop=(k == KD - 1))
            pu = ps1_p.tile([128, 512], F32, name="ps1")
            for k in range(KD):
                nc.tensor.matmul(
                    pu[:, :cs],
                    wu_sb[k][:, u_off + i * 128:u_off + (i + 1) * 128],
                    xin[k][:, xoff:xoff + cs],
                    start=(k == 0), stop=(k == KD - 1))
            hs = h_p.tile([128, 512], BF16, name="h")
            nc.scalar.activation(hs[:, :cs], pg[:, :cs], AF.Silu)
            hu = h_p.tile([128, 512], BF16, name="h")
            nc.vector.tensor_copy(hu[:, :cs], pu[:, :cs])
            nc.vector.tensor_tensor(
                inter[i][:, :cs], hs[:, :cs], hu[:, :cs], ALU.mult)
        for m2 in range(cs // 128):
            st = st_p.tile([128, D], BF16, name="st")
            for n2 in range(D // 512):
                ps2 = ps2_p.tile([128, 512], F32, name="ps2")
                for k2 in range(KI):
                    nc.tensor.matmul(
                        ps2[:], inter[k2][:, m2 * 128:(m2 + 1) * 128],
                        wd_sb[k2][:, n2 * 512:(n2 + 1) * 512],
                        start=(k2 == 0), stop=(k2 == KI - 1))
                if n2 % 2 == 0:
                    nc.scalar.copy(st[:, n2 * 512:(n2 + 1) * 512], ps2[:])
                else:
                    nc.vector.tensor_copy(st[:, n2 * 512:(n2 + 1) * 512], ps2[:])
            nc.sync.dma_start(
                out_d[obase + m2 * 128:obase + (m2 + 1) * 128, :], st[:])

    with tile.TileContext(nc) as tc:
        # ---------------- routed experts ----------------
        with (
            tc.tile_pool(name="wgu", bufs=20) as wgu_p,
            tc.tile_pool(name="wdn", bufs=8) as wdn_p,
            tc.tile_pool(name="xt", bufs=16) as xt_p,
            tc.tile_pool(name="h", bufs=3) as h_p,
            tc.tile_pool(name="inter", bufs=8) as i_p,
            tc.tile_pool(name="st", bufs=2) as st_p,
            tc.tile_pool(name="ps1", bufs=4, space="PSUM") as ps1_p,
            tc.tile_pool(name="ps2", bufs=3, space="PSUM") as ps2_p,
        ):
            for e in range(EPC):
                wgu_sb = []
                xt_sb = []
                for k in range(KD):
                    wt = wgu_p.tile([128, 2 * I], BF16, name="wgu")
                    nc.sync.dma_start(wt[:], wgu_d[e, k])
                    wgu_sb.append(wt)
                    xtt = xt_p.tile([128, C], BF16, name="xt")
                    nc.scalar.dma_start(xtt[:], xt_d[e, k])
                    xt_sb.append(xtt)
                wdn_sb = []
                for k in range(KI):
                    wt = wdn_p.tile([128, D], BF16, name="wdn")
                    nc.scalar.dma_start(wt[:], wdn_d[e, k])
                    wdn_sb.append(wt)
                cbase = 0
                for cs in _chunks(C):
                    swiglu_gemms(wgu_sb, 0, wgu_sb, I, xt_sb,
                                 cbase, cs, wdn_sb, y_d[e], cbase,
                                 i_p, h_p, ps1_p, ps2_p, st_p)
                    cbase += cs

        # ---------------- shared expert (data-parallel) ----------------
        with (
            tc.tile_pool(name="swg", bufs=16) as swg_p,
            tc.tile_pool(name="swu", bufs=16) as swu_p,
            tc.tile_pool(name="swd", bufs=8) as swd_p,
            tc.tile_pool(name="sxt", bufs=16) as sxt_p,
            tc.tile_pool(name="sh", bufs=3) as sh_p,
            tc.tile_pool(name="sinter", bufs=8) as si_p,
            tc.tile_pool(name="sst", bufs=2) as sst_p,
            tc.tile_pool(name="sps1", bufs=3, space="PSUM") as sps1_p,
            tc.tile_pool(name="sps2", bufs=2, space="PSUM") as sps2_p,
        ):
            wsg_sb, wsu_sb, xts_sb = [], [], []
            for k in range(KD):
                wt = swg_p.tile([128, I], BF16, name="swg")
                nc.sync.dma_start(wt[:], wsg_d[k])
                wsg_sb.append(wt)
                wt = swu_p.tile([128, I], BF16, name="swu")
                nc.sync.dma_start(wt[:], wsu_d[k])
                wsu_sb.append(wt)
                xtt = sxt_p.tile([128, TPC], BF16, name="sxt")
                nc.sync.dma_start(xtt[:], xts_d[k])
                xts_sb.append(xtt)
            wsd_sb = []
            for k in range(KI):
                wt = swd_p.tile([128, D], BF16, name="swd")
                nc.sync.dma_start(wt[:], wsd_d[k])
                wsd_sb.append(wt)
            for ch in range(TPC // 512):
                swiglu_gemms(wsg_sb, 0, wsu_sb, 0, xts_sb, ch * 512, 512,
                             wsd_sb, ys_d, ch * 512,
                             si_p, sh_p, sps1_p, sps2_p, sst_p)
    nc.compile()
    return nc


_BUILD_CACHE = {}


def _get_nc(C):
    if C not in _BUILD_CACHE:
        _BUILD_CACHE[C] = _build(C)
    return _BUILD_CACHE[C]


def _route(x_flat, gate_w, e_bias):
    """Replicates the reference router in numpy (f32)."""
    logits = x_flat @ gate_w                      # [N, E]
    scores = 1.0 / (1.0 + np.exp(-logits))
    sfr = scores + e_bias
    epg = E // N_GROUP
    grouped = sfr.reshape(N, N_GROUP, epg)
    top2 = np.partition(grouped, epg - 2, axis=2)[:, :, epg - 2:].sum(2)
    topg = np.argsort(-top2, axis=1, kind="stable")[:, :TOPK_GROUP]
    gmask = np.zeros((N, N_GROUP), bool)
    gmask[np.arange(N)[:, None], topg] = True
    emask = np.repeat(gmask, epg, axis=1)
    masked = np.where(emask, sfr, -np.inf)
    topk_idx = np.argsort(-masked, axis=1, kind="stable")[:, :TOP_K].astype(np.int32)
    topk_w = np.take_along_axis(scores, topk_idx, axis=1)
    topk_w = topk_w / (topk_w.sum(-1, keepdims=True) + 1e-20) * ROUTED_SCALE
    return topk_idx, topk_w, scores


def _prep_in_maps(x_flat, topk_idx, gate_up, down, shared_gate, shared_up,
                  shared_down):
    flat = topk_idx.reshape(-1).astype(np.int64)
    order = np.argsort(flat, kind="stable")
    counts = np.bincount(flat, minlength=E)
    starts = np.zeros(E + 1, np.int64)
    np.cumsum(counts, out=starts[1:])
    C = max(512, int(math.ceil(counts.max() / 128)) * 128)

    x_bf = x_flat.astype(bf16)
    wsg = np.ascontiguousarray(shared_gate.astype(bf16)).reshape(KD, 128, I)
    wsu = np.ascontiguousarray(shared_up.astype(bf16)).reshape(KD, 128, I)
    wsd = np.ascontiguousarray(shared_down.astype(bf16)).reshape(KI, 128, D)

    in_maps = []
    tids_all = []
    for c in range(NCORES):
        xt = np.zeros((EPC, KD, 128, C), bf16)
        for s in range(EPC):
            e = c * EPC + s
            tids = order[starts[e]:starts[e + 1]] // TOP_K
            tids_all.append(tids)
            xt[s].reshape(D, C)[:, :counts[e]] = x_bf[tids].T
        wgu = gate_up[c * EPC:(c + 1) * EPC].astype(bf16).reshape(
            EPC, KD, 128, 2 * I)
        wdn = down[c * EPC:(c + 1) * EPC].astype(bf16).reshape(EPC, KI, 128, D)
        xts = np.ascontiguousarray(x_bf[c * TPC:(c + 1) * TPC].T).reshape(
            KD, 128, TPC)
        in_maps.append({
            "xt": xt, "wgu": wgu, "wdn": wdn, "xts": xts,
            "wsg": wsg, "wsu": wsu, "wsd": wsd,
        })
    return in_maps, order, counts, starts, C


def kernel(x, gate_w, e_bias, gate_up, down, shared_gate, shared_up,
           shared_down):
    x = np.asarray(x, np.float32)
    gate_w = np.asarray(gate_w, np.float32)
    e_bias = np.asarray(e_bias, np.float32)
    gate_up = np.asarray(gate_up, np.float32)
    down = np.asarray(down, np.float32)
    shared_gate = np.asarray(shared_gate, np.float32)
    shared_up = np.asarray(shared_up, np.float32)
    shared_down = np.asarray(shared_down, np.float32)

    x_flat = x.reshape(N, D)
    topk_idx, topk_w, scores = _route(x_flat, gate_w, e_bias)
    in_maps, order, counts, starts, C = _prep_in_maps(
        x_flat, topk_idx, gate_up, down, shared_gate, shared_up, shared_down)

    nc = _get_nc(C)
    res = run_bass_kernel_spmd(nc, in_maps, core_ids=list(range(NCORES)))
    results = res.results

    sorted_out = np.empty((N * TOP_K, D), np.float32)
    for e in range(E):
        c, s = e // EPC, e % EPC
        sorted_out[starts[e]:starts[e + 1]] = results[c]["y"][s][:counts[e]]
    w_sorted = topk_w.reshape(-1)[order].astype(np.float32)
    sorted_out *= w_sorted[:, None]
    unsorted = np.empty_like(sorted_out)
    unsorted[order] = sorted_out
    routed = unsorted.reshape(N, TOP_K, D).sum(1)

    shared = np.concatenate(
        [results[c]["ys"].astype(np.float32) for c in range(NCORES)], 0)
    out = (routed + shared).reshape(B, S, D)
    return out, topk_idx, scores


# revision 12
# speedup vs baseline: 2.0491x; 1.0415x over previous
"""MoE block (nn_MoEBlock_40407052320888) on 8 Trainium2 NeuronCores.

Strategy (expert-parallel per the sharding hint):
- Router runs on host (tiny: 8192x2048x32 matmul = 0.1% of FLOPs); routing
  determines the shard layout, so it is part of input sharding.
- 32 experts sharded 4-per-core. Host gathers each expert's tokens
  (padded to capacity C), transposed to [D, C] so the device kernel needs
  zero on-chip transposes. Device does the heavy grouped SwiGLU GEMMs in
  bf16 with f32 PSUM accumulation.
- Shared-expert MLP is data-parallel: each core takes 1024 tokens.
- Host scatters routed outputs back (scatter-assign, weighted sum over K).
"""
import math
import sys

sys.path.insert(0, "/opt/trn_rl_repo")

import numpy as np
import ml_dtypes

import concourse.bacc as bacc
import concourse.mybir as mybir
import concourse.tile as tile
from concourse.bass_utils import run_bass_kernel_spmd

AF = mybir.ActivationFunctionType
ALU = mybir.AluOpType
BF16 = mybir.dt.bfloat16
F32 = mybir.dt.float32
bf16 = ml_dtypes.bfloat16

B, S, D, E, I = 4, 2048, 2048, 32, 1024
N = B * S
N_GROUP, TOPK_GROUP, TOP_K = 8, 4, 8
ROUTED_SCALE = 2.5
NCORES = 8
EPC = E // NCORES          # experts per core
TPC = N // NCORES          # tokens per core for the shared expert
KD = D // 128              # k-tiles over D
KI = I // 128              # k-tiles over I


def _chunks(C):
    out = [512] * (C // 512)
    if C % 512:
        out.append(C % 512)
    return out


def _build(C):
    """Bass program for one core: 4 experts (capacity C) + shared MLP."""
    nc = bacc.Bacc("TRN2", target_bir_lowering=False)
    xt_d = nc.dram_tensor("xt", [EPC, KD, 128, C], BF16, kind="ExternalInput")
    wgu_d = nc.dram_tensor("wgu", [EPC + 1, KD, 128, 2 * I], BF16,
                           kind="ExternalInput")
    wdn_d = nc.dram_tensor("wdn", [EPC + 1, KI, 128, D], BF16,
                           kind="ExternalInput")
    xts_d = nc.dram_tensor("xts", [KD, 128, TPC], BF16, kind="ExternalInput")
    y_d = nc.dram_tensor("y", [EPC, C, D], BF16, kind="ExternalOutput")
    ys_d = nc.dram_tensor("ys", [TPC, D], BF16, kind="ExternalOutput")

    def swiglu_gemms(wg_sb, g_off, wu_sb, u_off, xin, xoff, cs, wd_sb,
                     out_d, obase, i_p, h_p, ps1_p, ps2_p, st_p):
        """h = silu(Wg^T x)*(Wu^T x); out = h^T Wd ; writes [cs, D] to
        out_d[obase:obase+cs]. xin: list of KD sbuf tiles [128, >=xoff+cs]."""
        inter = [i_p.tile([128, 512], BF16, name="inter") for _ in range(KI)]
        for i in range(KI):
            pg = ps1_p.tile([128, 512], F32, name="ps1")
            for k in range(KD):
                nc.tensor.matmul(
                    pg[:, :cs],
                    wg_sb[k][:, g_off + i * 128:g_off + (i + 1) * 128],
                    xin[k][:, xoff:xoff + cs],
                    start=(k == 0), stop=(k == KD - 1))
            pu = ps1_p.tile([128, 512], F32, name="ps1")
            for k in range(KD):
                nc.tensor.matmul(
                    pu[:, :cs],
                    wu_sb[k][:, u_off + i * 128:u_off + (i + 1) * 128],
                    xin[k][:, xoff:xoff + cs],
                    start=(k == 0), stop=(k == KD - 1))
            hs = h_p.tile([128, 512], BF16, name="h")
            nc.scalar.activation(hs[:, :cs], pg[:, :cs], AF.Silu)
            hu = h_p.tile([128, 512], BF16, name="h")
            nc.vector.tensor_copy(hu[:, :cs], pu[:, :cs])
            nc.vector.tensor_tensor(
                inter[i][:, :cs], hs[:, :cs], hu[:, :cs], ALU.mult)
        for m2 in range(cs // 128):
            st = st_p.tile([128, D], BF16, name="st")
            for n2 in range(D // 512):
                ps2 = ps2_p.tile([128, 512], F32, name="ps2")
                for k2 in range(KI):
                    nc.tensor.matmul(
                        ps2[:], inter[k2][:, m2 * 128:(m2 + 1) * 128],
                        wd_sb[k2][:, n2 * 512:(n2 + 1) * 512],
                        start=(k2 == 0), stop=(k2 == KI - 1))
                if n2 % 2 == 0:
                    nc.scalar.copy(st[:, n2 * 512:(n2 + 1) * 512], ps2[:])
                else:
                    nc.vector.tensor_copy(st[:, n2 * 512:(n2 + 1) * 512], ps2[:])
            nc.sync.dma_start(
                out_d[obase + m2 * 128:obase + (m2 + 1) * 128, :], st[:])

    with tile.TileContext(nc) as tc:
        # ---------------- routed experts ----------------
        with (
            tc.tile_pool(name="wgu", bufs=20) as wgu_p,
            tc.tile_pool(name="wdn", bufs=8) as wdn_p,
            tc.tile_pool(name="xt", bufs=16) as xt_p,
            tc.tile_pool(name="h", bufs=3) as h_p,
            tc.tile_pool(name="inter", bufs=8) as i_p,
            tc.tile_pool(name="st", bufs=2) as st_p,
            tc.tile_pool(name="ps1", bufs=4, space="PSUM") as ps1_p,
            tc.tile_pool(name="ps2", bufs=3, space="PSUM") as ps2_p,
        ):
            for e in range(EPC + 1):
                shared = e == EPC
                cap = TPC if shared else C
                wgu_sb = []
                xt_sb = []
                for k in range(KD):
                    wt = wgu_p.tile([128, 2 * I], BF16, name="wgu")
                    nc.sync.dma_start(wt[:], wgu_d[e, k])
                    wgu_sb.append(wt)
                    xtt = xt_p.tile([128, C], BF16, name="xt")
                    if shared:
                        nc.scalar.dma_start(xtt[:, :cap], xts_d[k])
                    else:
                        nc.scalar.dma_start(xtt[:], xt_d[e, k])
                    xt_sb.append(xtt)
                wdn_sb = []
                for k in range(KI):
                    wt = wdn_p.tile([128, D], BF16, name="wdn")
                    nc.scalar.dma_start(wt[:], wdn_d[e, k])
                    wdn_sb.append(wt)
                out_d = ys_d if shared else y_d[e]
                cbase = 0
                for cs in _chunks(cap):
                    swiglu_gemms(wgu_sb, 0, wgu_sb, I, xt_sb,
                                 cbase, cs, wdn_sb, out_d, cbase,
                                 i_p, h_p, ps1_p, ps2_p, st_p)
                    cbase += cs
    nc.compile()
    return nc


_BUILD_CACHE = {}


def _get_nc(C):
    if C not in _BUILD_CACHE:
        _BUILD_CACHE[C] = _build(C)
    return _BUILD_CACHE[C]


def _route(x_flat, gate_w, e_bias):
    """Replicates the reference router in numpy (f32)."""
    logits = x_flat @ gate_w                      # [N, E]
    scores = 1.0 / (1.0 + np.exp(-logits))
    sfr = scores + e_bias
    epg = E // N_GROUP
    grouped = sfr.reshape(N, N_GROUP, epg)
    top2 = np.partition(grouped, epg - 2, axis=2)[:, :, epg - 2:].sum(2)
    topg = np.argsort(-top2, axis=1, kind="stable")[:, :TOPK_GROUP]
    gmask = np.zeros((N, N_GROUP), bool)
    gmask[np.arange(N)[:, None], topg] = True
    emask = np.repeat(gmask, epg, axis=1)
    masked = np.where(emask, sfr, -np.inf)
    topk_idx = np.argsort(-masked, axis=1, kind="stable")[:, :TOP_K].astype(np.int32)
    topk_w = np.take_along_axis(scores, topk_idx, axis=1)
    topk_w = topk_w / (topk_w.sum(-1, keepdims=True) + 1e-20) * ROUTED_SCALE
    return topk_idx, topk_w, scores


def _prep_in_maps(x_flat, topk_idx, gate_up, down, shared_gate, shared_up,
                  shared_down):
    flat = topk_idx.reshape(-1).astype(np.int64)
    order = np.argsort(flat, kind="stable")
    counts = np.bincount(flat, minlength=E)
    starts = np.zeros(E + 1, np.int64)
    np.cumsum(counts, out=starts[1:])
    C = max(512, int(math.ceil(counts.max() / 128)) * 128)

    x_bf = x_flat.astype(bf16)
    wsgu = np.concatenate([shared_gate, shared_up], axis=1).astype(bf16).reshape(
        1, KD, 128, 2 * I)
    wsd = shared_down.astype(bf16).reshape(1, KI, 128, D)

    in_maps = []
    for c in range(NCORES):
        xt = np.zeros((EPC, KD, 128, C), bf16)
        for s in range(EPC):
            e = c * EPC + s
            tids = order[starts[e]:starts[e + 1]] // TOP_K
            xt[s].reshape(D, C)[:, :counts[e]] = x_bf[tids].T
        wgu = np.concatenate([
            gate_up[c * EPC:(c + 1) * EPC].astype(bf16).reshape(
                EPC, KD, 128, 2 * I), wsgu], 0)
        wdn = np.concatenate([
            down[c * EPC:(c + 1) * EPC].astype(bf16).reshape(
                EPC, KI, 128, D), wsd], 0)
        xts = np.ascontiguousarray(x_bf[c * TPC:(c + 1) * TPC].T).reshape(
            KD, 128, TPC)
        in_maps.append({"xt": xt, "wgu": wgu, "wdn": wdn, "xts": xts})
    return in_maps, order, counts, starts, C


def kernel(x, gate_w, e_bias, gate_up, down, shared_gate, shared_up,
           shared_down):
    x = np.asarray(x, np.float32)
    gate_w = np.asarray(gate_w, np.float32)
    e_bias = np.asarray(e_bias, np.float32)
    gate_up = np.asarray(gate_up, np.float32)
    down = np.asarray(down, np.float32)
    shared_gate = np.asarray(shared_gate, np.float32)
    shared_up = np.asarray(shared_up, np.float32)
    shared_down = np.asarray(shared_down, np.float32)

    x_flat = x.reshape(N, D)
    topk_idx, topk_w, scores = _route(x_flat, gate_w, e_bias)
    in_maps, order, counts, starts, C = _prep_in_maps(
        x_flat, topk_idx, gate_up, down, shared_gate, shared_up, shared_down)

    nc = _get_nc(C)
    res = run_bass_kernel_spmd(nc, in_maps, core_ids=list(range(NCORES)))
    results = res.results

    sorted_out = np.empty((N * TOP_K, D), np.float32)
    for e in range(E):
        c, s = e // EPC, e % EPC
        sorted_out[starts[e]:starts[e + 1]] = results[c]["y"][s][:counts[e]]
    w_sorted = topk_w.reshape(-1)[order].astype(np.float32)
    sorted_out *= w_sorted[:, None]
    unsorted = np.empty_like(sorted_out)
    unsorted[order] = sorted_out
    routed = unsorted.reshape(N, TOP_K, D).sum(1)

    shared = np.concatenate(
        [results[c]["ys"].astype(np.float32) for c in range(NCORES)], 0)
    out = (routed + shared).reshape(B, S, D)
    return out, topk_idx, scores


# revision 13
# speedup vs baseline: 2.0575x; 1.0041x over previous
"""MoE block (nn_MoEBlock_40407052320888) on 8 Trainium2 NeuronCores.

Strategy (expert-parallel per the sharding hint):
- Router runs on host (tiny: 8192x2048x32 matmul = 0.1% of FLOPs); routing
  determines the shard layout, so it is part of input sharding.
- 32 experts sharded 4-per-core. Host gathers each expert's tokens
  (padded to capacity C), transposed to [D, C] so the device kernel needs
  zero on-chip transposes. Device does the heavy grouped SwiGLU GEMMs in
  bf16 with f32 PSUM accumulation.
- Shared-expert MLP is data-parallel: each core takes 1024 tokens.
- Host scatters routed outputs back (scatter-assign, weighted sum over K).
"""
import math
import sys

sys.path.insert(0, "/opt/trn_rl_repo")

import numpy as np
import ml_dtypes

import concourse.bacc as bacc
import concourse.mybir as mybir
import concourse.tile as tile
from concourse.bass_utils import run_bass_kernel_spmd

AF = mybir.ActivationFunctionType
ALU = mybir.AluOpType
BF16 = mybir.dt.bfloat16
F32 = mybir.dt.float32
bf16 = ml_dtypes.bfloat16

B, S, D, E, I = 4, 2048, 2048, 32, 1024
N = B * S
N_GROUP, TOPK_GROUP, TOP_K = 8, 4, 8
ROUTED_SCALE = 2.5
NCORES = 8
EPC = E // NCORES          # experts per core
TPC = N // NCORES          # tokens per core for the shared expert
KD = D // 128              # k-tiles over D
KI = I // 128              # k-tiles over I


def _chunks(C):
    out = [512] * (C // 512)
    if C % 512:
        out.append(C % 512)
    return out


def _build(C):
    """Bass program for one core: 4 experts (capacity C) + shared MLP."""
    nc = bacc.Bacc("TRN2", target_bir_lowering=False)
    xt_d = nc.dram_tensor("xt", [EPC, KD, 128, C], BF16, kind="ExternalInput")
    wgu_d = nc.dram_tensor("wgu", [EPC + 1, KD, 128, 2 * I], BF16,
                           kind="ExternalInput")
    wdn_d = nc.dram_tensor("wdn", [EPC + 1, KI, 128, D], BF16,
                           kind="ExternalInput")
    xts_d = nc.dram_tensor("xts", [KD, 128, TPC], BF16, kind="ExternalInput")
    y_d = nc.dram_tensor("y", [EPC, C, D], BF16, kind="ExternalOutput")
    ys_d = nc.dram_tensor("ys", [TPC, D], BF16, kind="ExternalOutput")

    def swiglu_gemms(wg_sb, g_off, wu_sb, u_off, xin, xoff, cs, wd_sb,
                     out_d, obase, i_p, h_p, ps1_p, ps2_p, st_p):
        """h = silu(Wg^T x)*(Wu^T x); out = h^T Wd ; writes [cs, D] to
        out_d[obase:obase+cs]. xin: list of KD sbuf tiles [128, >=xoff+cs]."""
        inter = [i_p.tile([128, 512], BF16, name="inter") for _ in range(KI)]
        for i in range(KI):
            pg = ps1_p.tile([128, 512], F32, name="ps1")
            for k in range(KD):
                nc.tensor.matmul(
                    pg[:, :cs],
                    wg_sb[k][:, g_off + i * 128:g_off + (i + 1) * 128],
                    xin[k][:, xoff:xoff + cs],
                    start=(k == 0), stop=(k == KD - 1))
            pu = ps1_p.tile([128, 512], F32, name="ps1")
            for k in range(KD):
                nc.tensor.matmul(
                    pu[:, :cs],
                    wu_sb[k][:, u_off + i * 128:u_off + (i + 1) * 128],
                    xin[k][:, xoff:xoff + cs],
                    start=(k == 0), stop=(k == KD - 1))
            hs = h_p.tile([128, 512], BF16, name="h")
            nc.scalar.activation(hs[:, :cs], pg[:, :cs], AF.Silu)
            hu = h_p.tile([128, 512], BF16, name="h")
            nc.vector.tensor_copy(hu[:, :cs], pu[:, :cs])
            nc.vector.tensor_tensor(
                inter[i][:, :cs], hs[:, :cs], hu[:, :cs], ALU.mult)
        for m2 in range(cs // 128):
            st = st_p.tile([128, D], BF16, name="st")
            for n2 in range(D // 512):
                ps2 = ps2_p.tile([128, 512], F32, name="ps2")
                for k2 in range(KI):
                    nc.tensor.matmul(
                        ps2[:], inter[k2][:, m2 * 128:(m2 + 1) * 128],
                        wd_sb[k2][:, n2 * 512:(n2 + 1) * 512],
                        start=(k2 == 0), stop=(k2 == KI - 1))
                nc.vector.tensor_copy(st[:, n2 * 512:(n2 + 1) * 512], ps2[:])
            nc.sync.dma_start(
                out_d[obase + m2 * 128:obase + (m2 + 1) * 128, :], st[:])

    with tile.TileContext(nc) as tc:
        # ---------------- routed experts ----------------
        with (
            tc.tile_pool(name="wgu", bufs=22) as wgu_p,
            tc.tile_pool(name="wdn", bufs=8) as wdn_p,
            tc.tile_pool(name="xt", bufs=16) as xt_p,
            tc.tile_pool(name="h", bufs=3) as h_p,
            tc.tile_pool(name="inter", bufs=8) as i_p,
            tc.tile_pool(name="st", bufs=2) as st_p,
            tc.tile_pool(name="ps1", bufs=5, space="PSUM") as ps1_p,
            tc.tile_pool(name="ps2", bufs=3, space="PSUM") as ps2_p,
        ):
            for e in range(EPC + 1):
                shared = e == EPC
                cap = TPC if shared else C
                wgu_sb = []
                xt_sb = []
                for k in range(KD):
                    wt = wgu_p.tile([128, 2 * I], BF16, name="wgu")
                    nc.sync.dma_start(wt[:], wgu_d[e, k])
                    wgu_sb.append(wt)
                    xtt = xt_p.tile([128, C], BF16, name="xt")
                    if shared:
                        nc.scalar.dma_start(xtt[:, :cap], xts_d[k])
                    else:
                        nc.scalar.dma_start(xtt[:], xt_d[e, k])
                    xt_sb.append(xtt)
                wdn_sb = []
                for k in range(KI):
                    wt = wdn_p.tile([128, D], BF16, name="wdn")
                    nc.scalar.dma_start(wt[:], wdn_d[e, k])
                    wdn_sb.append(wt)
                out_d = ys_d if shared else y_d[e]
                cbase = 0
                for cs in _chunks(cap):
                    swiglu_gemms(wgu_sb, 0, wgu_sb, I, xt_sb,
                                 cbase, cs, wdn_sb, out_d, cbase,
                                 i_p, h_p, ps1_p, ps2_p, st_p)
                    cbase += cs
    nc.compile()
    return nc


_BUILD_CACHE = {}


def _get_nc(C):
    if C not in _BUILD_CACHE:
        _BUILD_CACHE[C] = _build(C)
    return _BUILD_CACHE[C]


def _route(x_flat, gate_w, e_bias):
    """Replicates the reference router in numpy (f32)."""
    logits = x_flat @ gate_w                      # [N, E]
    scores = 1.0 / (1.0 + np.exp(-logits))
    sfr = scores + e_bias
    epg = E // N_GROUP
    grouped = sfr.reshape(N, N_GROUP, epg)
    top2 = np.partition(grouped, epg - 2, axis=2)[:, :, epg - 2:].sum(2)
    topg = np.argsort(-top2, axis=1, kind="stable")[:, :TOPK_GROUP]
    gmask = np.zeros((N, N_GROUP), bool)
    gmask[np.arange(N)[:, None], topg] = True
    emask = np.repeat(gmask, epg, axis=1)
    masked = np.where(emask, sfr, -np.inf)
    topk_idx = np.argsort(-masked, axis=1, kind="stable")[:, :TOP_K].astype(np.int32)
    topk_w = np.take_along_axis(scores, topk_idx, axis=1)
    topk_w = topk_w / (topk_w.sum(-1, keepdims=True) + 1e-20) * ROUTED_SCALE
    return topk_idx, topk_w, scores


def _prep_in_maps(x_flat, topk_idx, gate_up, down, shared_gate, shared_up,
                  shared_down):
    flat = topk_idx.reshape(-1).astype(np.int64)
    order = np.argsort(flat, kind="stable")
    counts = np.bincount(flat, minlength=E)
    starts = np.zeros(E + 1, np.int64)
    np.cumsum(counts, out=starts[1:])
    C = max(512, int(math.ceil(counts.max() / 128)) * 128)

    x_bf = x_flat.astype(bf16)
    wsgu = np.concatenate([shared_gate, shared_up], axis=1).astype(bf16).reshape(
        1, KD, 128, 2 * I)
    wsd = shared_down.astype(bf16).reshape(1, KI, 128, D)

    in_maps = []
    for c in range(NCORES):
        xt = np.zeros((EPC, KD, 128, C), bf16)
        for s in range(EPC):
            e = c * EPC + s
            tids = order[starts[e]:starts[e + 1]] // TOP_K
            xt[s].reshape(D, C)[:, :counts[e]] = x_bf[tids].T
        wgu = np.concatenate([
            gate_up[c * EPC:(c + 1) * EPC].astype(bf16).reshape(
                EPC, KD, 128, 2 * I), wsgu], 0)
        wdn = np.concatenate([
            down[c * EPC:(c + 1) * EPC].astype(bf16).reshape(
                EPC, KI, 128, D), wsd], 0)
        xts = np.ascontiguousarray(x_bf[c * TPC:(c + 1) * TPC].T).reshape(
            KD, 128, TPC)
        in_maps.append({"xt": xt, "wgu": wgu, "wdn": wdn, "xts": xts})
    return in_maps, order, counts, starts, C


def kernel(x, gate_w, e_bias, gate_up, down, shared_gate, shared_up,
           shared_down):
    x = np.asarray(x, np.float32)
    gate_w = np.asarray(gate_w, np.float32)
    e_bias = np.asarray(e_bias, np.float32)
    gate_up = np.asarray(gate_up, np.float32)
    down = np.asarray(down, np.float32)
    shared_gate = np.asarray(shared_gate, np.float32)
    shared_up = np.asarray(shared_up, np.float32)
    shared_down = np.asarray(shared_down, np.float32)

    x_flat = x.reshape(N, D)
    topk_idx, topk_w, scores = _route(x_flat, gate_w, e_bias)
    in_maps, order, counts, starts, C = _prep_in_maps(
        x_flat, topk_idx, gate_up, down, shared_gate, shared_up, shared_down)

    nc = _get_nc(C)
    res = run_bass_kernel_spmd(nc, in_maps, core_ids=list(range(NCORES)))
    results = res.results

    sorted_out = np.empty((N * TOP_K, D), np.float32)
    for e in range(E):
        c, s = e // EPC, e % EPC
        sorted_out[starts[e]:starts[e + 1]] = results[c]["y"][s][:counts[e]]
    w_sorted = topk_w.reshape(-1)[order].astype(np.float32)
    sorted_out *= w_sorted[:, None]
    unsorted = np.empty_like(sorted_out)
    unsorted[order] = sorted_out
    routed = unsorted.reshape(N, TOP_K, D).sum(1)

    shared = np.concatenate(
        [results[c]["ys"].astype(np.float32) for c in range(NCORES)], 0)
    out = (routed + shared).reshape(B, S, D)
    return out, topk_idx, scores
